# revision 13
# baseline (speedup 1.0000x reference)
"""SimCLR contrastive loss (NT-Xent) on 8 Trainium2 NeuronCores.

Reference computation (see problem):
    z  = concat(z_i, z_j)                     # [N, D], N = 8192, D = 256
    zn = z / max(||z||_row, eps)
    sim = zn @ zn.T / TEMP                    # TEMP = 0.5
    lse = logsumexp(sim with -inf diagonal, axis=1)
    pos[r] = sim[r, (r + B) mod N]
    loss = sum(lse - pos) / N

Distribution strategy (data parallel, mirrors world_size>1 SimCLR):
  Each core owns N/8 = 1024 rows of the similarity matrix and computes its
  [1024, 8192] block of logits against all of zn, reducing each row with a
  fused exp+rowsum on the Scalar engine.  The host passes each core the
  *column-rotated* transposed embedding matrix zT_c = roll(z.T, -1024*c,
  axis=1) so that one SPMD program works for every core: core c's own rows
  are always local columns [0, 1024), the positive-pair window for row-tile
  m is always local columns [N/2 + 128m, N/2 + 128m + 128), and the diagonal
  element is handled by subtracting the constant e^(1/TEMP) from the row sum
  (sim[i,i] = 1/TEMP up to fp rounding; the error this introduces is ~1e-10
  relative on the row sum).

Per-core kernel pipeline:
  1. DMA zT [256, 8192] fp32 into SBUF (two 128-partition tiles).
  2. sq = zT*zT (VectorE, bf16 out); column sums-of-squares via a ones-vector
     matmul (TensorE reduces along partitions); inv = exp(-0.5*ln(ss))
     computed on ScalarE in the [128, G/128] layout reached via a tiny DRAM
     round-trip (exp/ln share one ACT table set; the Rsqrt LUT is banned for
     accuracy).  inv is broadcast to all 128 partitions with a
     partition-stride-0 DMA and applied: znb = zT * inv_bcast, cast to bf16
     (bf16 matmul runs 4x faster than fp32 on the PE; fp32 PSUM accumulate
     keeps the row sums accurate).
  3. Main loop (sweep-major so the PE never waits on the prologue): for each
     2048-column sweep and each 128-row tile: 8 matmuls fill a 4-bank PSUM
     tile with cosine similarities; the positive-pair diagonal window is
     extracted with a fused multiply-by-identity row-reduce (scale=2.0 folds
     in 1/TEMP); ScalarE then computes exp(2*sim) in place with a fused
     row-sum (accum_out).
  4. lse = ln(rowsum - e^2); per-row output (lse - pos) lands in a [128, 8]
     tile DMA'd out; the host sums everything in fp64 and divides by N.
"""

import os
import sys

import numpy as np

B = 4096
D = 256
N = 2 * B
NCORES = 8
RPC = N // NCORES  # rows per core

_CANDIDATE_PATHS = ("/opt/trn_rl_repo", "/root/.axon_site/_ro/trn_rl_repo")


def _ensure_import_path():
    try:
        import concourse.bass  # noqa: F401
        return
    except ImportError:
        pass
    for p in _CANDIDATE_PATHS:
        if os.path.isdir(p) and p not in sys.path:
            sys.path.insert(0, p)
    import concourse.bass  # noqa: F401


def build_program(n=N, d=D, rpc=RPC):
    """Build and compile the single SPMD Bass program shared by all cores."""
    _ensure_import_path()
    from contextlib import ExitStack

    import concourse.bacc as bacc
    import concourse.tile as tile
    from concourse import mybir

    f32 = mybir.dt.float32
    bf16 = mybir.dt.bfloat16
    FT = mybir.ActivationFunctionType
    OP = mybir.AluOpType

    P = 128
    CH = 512                       # matmul free dim = one fp32 PSUM bank
    kt = (d + P - 1) // P          # contraction tiles over embedding dim
    mt = rpc // P                  # row tiles per core
    nch = n // CH
    swch = min(4, nch)             # chunks per PSUM sweep (4 banks)
    nsw = nch // swch              # sweeps (also the prologue column groups)
    GW = swch * CH                 # group/sweep width in columns
    EXP2 = float(np.exp(2.0))      # exp(sim[i,i]) = exp(1/TEMP)

    assert rpc % P == 0 and n % CH == 0 and nch % swch == 0 and GW % P == 0
    # the positive window must sit inside a single sweep tile
    assert (n // 2) % GW == 0 or ((n // 2 + rpc) - 1) // GW == (n // 2) // GW

    nc = bacc.Bacc("TRN2", target_bir_lowering=False, debug=False)
    zT_d = nc.dram_tensor("zT", [d, n], f32, kind="ExternalInput").ap()
    id_d = nc.dram_tensor("ident", [P, P], f32, kind="ExternalInput").ap()
    out_d = nc.dram_tensor("out", [P, mt], f32, kind="ExternalOutput").ap()
    invd = nc.dram_tensor("invd", [1, n], f32).ap()  # scratch: 1/norm reshape

    with tile.TileContext(nc) as tc, ExitStack() as ctx:
        big = ctx.enter_context(tc.tile_pool(name="big", bufs=1))
        sqp = ctx.enter_context(tc.tile_pool(name="sqp", bufs=2))
        small = ctx.enter_context(tc.tile_pool(name="small", bufs=2))
        stat = ctx.enter_context(tc.tile_pool(name="stat", bufs=1))
        mps = ctx.enter_context(tc.tile_pool(name="mps", bufs=2, space="PSUM"))

        pdims = [min(P, d - P * k) for k in range(kt)]
        zt = [big.tile([pdims[k], n], f32, tag=f"zt{k}", name=f"zt{k}") for k in range(kt)]
        znb = [big.tile([pdims[k], n], bf16, tag=f"znb{k}", name=f"znb{k}") for k in range(kt)]
        ident_sb = stat.tile([P, P], f32, tag="ident")
        ones_sb = stat.tile([P, 1], bf16, tag="ones")
        ones_row = stat.tile([1, P], f32, tag="ones_row")
        out_sb = stat.tile([P, mt], f32, tag="out_sb")
        partials = stat.tile([P, mt, nsw], f32, tag="partials")
        poss = stat.tile([P, mt], f32, tag="poss")

        nc.sync.dma_start(out=ident_sb, in_=id_d)
        nc.vector.memset(ones_sb, 1.0)
        nc.vector.memset(ones_row, 1.0)

        tgw = GW // P
        i32 = mybir.dt.int32

        def prologue_group(g):
            """Load cols [GW*g, GW*(g+1)), compute normalized bf16 znb.
            The whole 1/sqrt chain stays on-chip: ss lands in [128, tgw]
            PSUM via per-block ones-matmuls, DVE runs a Newton rsqrt from
            the int bit-trick seed (no ACT -> the exp table stays
            resident), PE transposes inv and broadcasts it to a [128, GW]
            PSUM tile with rank-1 matmuls."""
            G = slice(GW * g, GW * (g + 1))
            for k in range(kt):
                nc.sync.dma_start(
                    out=zt[k][:, G], in_=zT_d[P * k : P * k + pdims[k], G]
                )
            sqs = [
                sqp.tile([pdims[k], GW], bf16, tag=f"sq{k}", name=f"sq{k}")
                for k in range(kt)
            ]
            for k in range(kt):
                nc.vector.tensor_mul(sqs[k], zt[k][:, G], zt[k][:, G])
            # dance tile: bank 0 cols [0, tgw) = ss, bank 1 holds inv^T
            dance = mps.tile([P, GW], f32, tag="ps", name="dance")
            for t in range(tgw):
                for k in range(kt):
                    nc.tensor.matmul(
                        dance[:, t : t + 1],
                        sqs[k][:, P * t : P * (t + 1)],
                        ones_sb[: pdims[k]],
                        start=(k == 0),
                        stop=(k == kt - 1),
                    )
            ss = dance[:, 0:tgw]
            # inv = 1/sqrt(ss): Quake seed y0 = bits(0x5f3759df - (i >> 1))
            # then two Newton steps y *= 1.5 - 0.5*ss*y^2  (rel err ~5e-6)
            ii = small.tile([P, tgw], i32, tag="ii")
            nc.vector.tensor_scalar(
                out=ii, in0=ss.bitcast(i32), scalar1=1, scalar2=None,
                op0=OP.arith_shift_right,
            )
            # K - t computed as (t ^ -1) + (K + 1); walrus disallows mixing
            # bitwise and arithmetic ops in one TENSOR_SCALAR, so two insts
            nc.vector.tensor_scalar(
                out=ii, in0=ii, scalar1=-1, scalar2=None, op0=OP.bitwise_xor
            )
            nc.vector.tensor_scalar(
                out=ii, in0=ii, scalar1=0x5F3759DF + 1, scalar2=None, op0=OP.add
            )
            y = ii.bitcast(f32)
            t_ = small.tile([P, tgw], f32, tag="t_")
            for _ in range(2):
                nc.vector.tensor_mul(t_, y, y)
                nc.vector.tensor_mul(t_, t_, ss)
                nc.vector.tensor_scalar(
                    out=t_, in0=t_, scalar1=-0.5, scalar2=1.5,
                    op0=OP.mult, op1=OP.add,
                )
                nc.vector.tensor_mul(y, y, t_)
            # reshape inv [128, tgw] -> [1, GW] through DRAM (SBUF APs
            # cannot cross partitions) so the rank-1 broadcast matmuls can
            # read it from partition 0
            nc.sync.dma_start(
                out=invd[0, G].rearrange("(t p) -> p t", p=P), in_=y
            )
            invR = small.tile([1, GW], f32, tag="invR")
            nc.sync.dma_start(out=invR, in_=invd[:, G])
            # broadcast inv along partitions: rank-1 ones x inv matmuls
            bcast = mps.tile([P, GW], f32, tag="ps", name="bcast")
            for t in range(tgw):
                nc.tensor.matmul(
                    bcast[:, P * t : P * (t + 1)],
                    ones_row,
                    invR[0:1, P * t : P * (t + 1)],
                    start=True,
                    stop=True,
                )
            for k in range(kt):
                nc.vector.tensor_mul(
                    znb[k][:, G], zt[k][:, G], bcast[: pdims[k], :]
                )

        def main_sweep(s):
            for m in range(mt):
                ps = mps.tile([P, GW], f32, tag="ps", name="ps")
                for k in range(kt):
                    for c in range(swch):
                        cols = slice(GW * s + CH * c, GW * s + CH * (c + 1))
                        nc.tensor.matmul(
                            ps[:, CH * c : CH * (c + 1)],
                            znb[k][:, P * m : P * (m + 1)],
                            znb[k][:, cols],
                            start=(k == 0),
                            stop=(k == kt - 1),
                        )
                w0 = n // 2 + P * m  # positive-pair window (local cols)
                if w0 // GW == s:
                    off = w0 % GW
                    junk = small.tile([P, P], f32, tag="junk")
                    nc.vector.scalar_tensor_tensor(
                        out=junk,
                        in0=ps[:, off : off + P],
                        scalar=2.0,
                        in1=ident_sb,
                        op0=OP.mult,
                        op1=OP.mult,
                        accum_out=poss[:, m : m + 1],
                    )
                nc.scalar.activation(
                    out=ps,
                    in_=ps,
                    func=FT.Exp,
                    scale=2.0,
                    accum_out=partials[:, m, s : s + 1],
                )

        # Interleave: each group's prologue is emitted two sweeps ahead so
        # no engine's in-order stream stalls on a later group's chain.
        for g in range(min(2, nsw)):
            prologue_group(g)
        for s in range(nsw):
            main_sweep(s)
            if s + 2 < nsw:
                prologue_group(s + 2)

        # ---- Per-row finalization ----
        for m in range(mt):
            S = small.tile([P, 1], f32, tag="S")
            nc.vector.tensor_reduce(
                out=S,
                in_=partials[:, m, :],
                axis=mybir.AxisListType.X,
                op=OP.add,
            )
            nc.vector.tensor_scalar_add(S, S, -EXP2)
            lse = small.tile([P, 1], f32, tag="lse")
            nc.scalar.activation(out=lse, in_=S, func=FT.Ln)
            nc.vector.tensor_tensor(
                out=out_sb[:, m : m + 1],
                in0=lse,
                in1=poss[:, m : m + 1],
                op=OP.subtract,
            )
        nc.sync.dma_start(out=out_d, in_=out_sb)

    nc.compile()
    return nc


def make_in_maps(z_i, z_j, n=N, rpc=RPC, ncores=NCORES):
    """Host-side sharding: rotated transposed embeddings per core."""
    z = np.concatenate(
        [np.asarray(z_i, dtype=np.float32), np.asarray(z_j, dtype=np.float32)],
        axis=0,
    )
    zT = np.ascontiguousarray(z.T)  # [D, N]
    ident = np.eye(128, dtype=np.float32)
    in_maps = []
    for c in range(ncores):
        zT_c = np.ascontiguousarray(np.roll(zT, -rpc * c, axis=1))
        in_maps.append({"zT": zT_c, "ident": ident})
    return in_maps


def gather_loss(results, n=N):
    """Host-side unshard: fp64 sum of all per-row (lse - pos) values / N."""
    total = 0.0
    for r in results:
        total += np.asarray(r["out"], dtype=np.float64).sum()
    return np.float32(total / n)


_PROGRAM_CACHE = {}


def kernel(z_i, z_j):
    _ensure_import_path()
    from concourse.bass_utils import run_bass_kernel_spmd

    key = (N, D, RPC)
    if key not in _PROGRAM_CACHE:
        _PROGRAM_CACHE[key] = build_program()
    nc = _PROGRAM_CACHE[key]
    in_maps = make_in_maps(z_i, z_j)
    results = run_bass_kernel_spmd(nc, in_maps, list(range(NCORES))).results
    return gather_loss(results)


if __name__ == "__main__":
    rng = np.random.default_rng(0)
    z_i = rng.standard_normal((B, D), dtype=np.float32)
    z_j = rng.standard_normal((B, D), dtype=np.float32)
    loss = kernel(z_i, z_j)
    print("loss:", loss)


# revision 14
# speedup vs baseline: 1.3124x; 1.3124x over previous
"""SimCLR contrastive loss (NT-Xent) on 8 Trainium2 NeuronCores.

Reference computation (see problem):
    z  = concat(z_i, z_j)                     # [N, D], N = 8192, D = 256
    zn = z / max(||z||_row, eps)
    sim = zn @ zn.T / TEMP                    # TEMP = 0.5
    lse = logsumexp(sim with -inf diagonal, axis=1)
    pos[r] = sim[r, (r + B) mod N]
    loss = sum(lse - pos) / N

Distribution strategy (data parallel, mirrors world_size>1 SimCLR):
  Each core owns N/8 = 1024 rows of the similarity matrix and computes its
  [1024, 8192] block of logits against all of zn, reducing each row with a
  fused exp+rowsum on the Scalar engine.  The host passes each core the
  *column-rotated* transposed embedding matrix zT_c = roll(z.T, -1024*c,
  axis=1) so that one SPMD program works for every core: core c's own rows
  are always local columns [0, 1024), the positive-pair window for row-tile
  m is always local columns [N/2 + 128m, N/2 + 128m + 128), and the diagonal
  element is handled by subtracting the constant e^(1/TEMP) from the row sum
  (sim[i,i] = 1/TEMP up to fp rounding; the error this introduces is ~1e-10
  relative on the row sum).

Per-core kernel pipeline:
  1. DMA zT [256, 8192] fp32 into SBUF (two 128-partition tiles).
  2. sq = zT*zT (VectorE, bf16 out); column sums-of-squares via a ones-vector
     matmul (TensorE reduces along partitions); inv = exp(-0.5*ln(ss))
     computed on ScalarE in the [128, G/128] layout reached via a tiny DRAM
     round-trip (exp/ln share one ACT table set; the Rsqrt LUT is banned for
     accuracy).  inv is broadcast to all 128 partitions with a
     partition-stride-0 DMA and applied: znb = zT * inv_bcast, cast to bf16
     (bf16 matmul runs 4x faster than fp32 on the PE; fp32 PSUM accumulate
     keeps the row sums accurate).
  3. Main loop (sweep-major so the PE never waits on the prologue): for each
     2048-column sweep and each 128-row tile: 8 matmuls fill a 4-bank PSUM
     tile with cosine similarities; the positive-pair diagonal window is
     extracted with a fused multiply-by-identity row-reduce (scale=2.0 folds
     in 1/TEMP); ScalarE then computes exp(2*sim) in place with a fused
     row-sum (accum_out).
  4. lse = ln(rowsum - e^2); per-row output (lse - pos) lands in a [128, 8]
     tile DMA'd out; the host sums everything in fp64 and divides by N.
"""

import os
import sys

import numpy as np

B = 4096
D = 256
N = 2 * B
NCORES = 8
RPC = N // NCORES  # rows per core

_CANDIDATE_PATHS = ("/opt/trn_rl_repo", "/root/.axon_site/_ro/trn_rl_repo")


def _ensure_import_path():
    try:
        import concourse.bass  # noqa: F401
        return
    except ImportError:
        pass
    for p in _CANDIDATE_PATHS:
        if os.path.isdir(p) and p not in sys.path:
            sys.path.insert(0, p)
    import concourse.bass  # noqa: F401


def build_program(n=N, d=D, rpc=RPC):
    """Build and compile the single SPMD Bass program shared by all cores."""
    _ensure_import_path()
    from contextlib import ExitStack

    import concourse.bacc as bacc
    import concourse.tile as tile
    from concourse import mybir

    f32 = mybir.dt.float32
    bf16 = mybir.dt.bfloat16
    FT = mybir.ActivationFunctionType
    OP = mybir.AluOpType

    P = 128
    CH = 512                       # matmul free dim = one fp32 PSUM bank
    kt = (d + P - 1) // P          # contraction tiles over embedding dim
    mt = rpc // P                  # row tiles per core
    nch = n // CH
    swch = min(4, nch)             # chunks per PSUM sweep (4 banks)
    nsw = nch // swch              # sweeps (also the prologue column groups)
    GW = swch * CH                 # group/sweep width in columns
    EXP2 = float(np.exp(2.0))      # exp(sim[i,i]) = exp(1/TEMP)

    assert rpc % P == 0 and n % CH == 0 and nch % swch == 0 and GW % P == 0
    # the positive window must sit inside a single sweep tile
    assert (n // 2) % GW == 0 or ((n // 2 + rpc) - 1) // GW == (n // 2) // GW

    nc = bacc.Bacc("TRN2", target_bir_lowering=False, debug=False)
    zT_d = nc.dram_tensor("zT", [d, n], f32, kind="ExternalInput").ap()
    id_d = nc.dram_tensor("ident", [P, P], f32, kind="ExternalInput").ap()
    out_d = nc.dram_tensor("out", [P, mt], f32, kind="ExternalOutput").ap()
    ssd = nc.dram_tensor("ssd", [1, n], f32).ap()    # scratch: col sumsq
    invd = nc.dram_tensor("invd", [1, n], f32).ap()  # scratch: 1/norm

    with tile.TileContext(nc) as tc, ExitStack() as ctx:
        big = ctx.enter_context(tc.tile_pool(name="big", bufs=1))
        sqp = ctx.enter_context(tc.tile_pool(name="sqp", bufs=2))
        bcp = ctx.enter_context(tc.tile_pool(name="bcp", bufs=2))
        small = ctx.enter_context(tc.tile_pool(name="small", bufs=2))
        stat = ctx.enter_context(tc.tile_pool(name="stat", bufs=1))
        mps = ctx.enter_context(tc.tile_pool(name="mps", bufs=2, space="PSUM"))

        pdims = [min(P, d - P * k) for k in range(kt)]
        zt = [big.tile([pdims[k], n], f32, tag=f"zt{k}", name=f"zt{k}") for k in range(kt)]
        znb = [big.tile([pdims[k], n], bf16, tag=f"znb{k}", name=f"znb{k}") for k in range(kt)]
        ident_sb = stat.tile([P, P], f32, tag="ident")
        ones_sb = stat.tile([P, 1], bf16, tag="ones")
        out_sb = stat.tile([P, mt], f32, tag="out_sb")
        partials = stat.tile([P, mt, nsw], f32, tag="partials")
        poss = stat.tile([P, mt], f32, tag="poss")

        nc.sync.dma_start(out=ident_sb, in_=id_d)
        nc.vector.memset(ones_sb, 1.0)

        tgw = GW // P
        i32 = mybir.dt.int32

        def prologue_group(g):
            """Load cols [GW*g, GW*(g+1)), compute normalized bf16 znb.
            The whole 1/sqrt chain stays on-chip: ss lands in [128, tgw]
            PSUM via per-block ones-matmuls, DVE runs a Newton rsqrt from
            the int bit-trick seed (no ACT -> the exp table stays
            resident), PE transposes inv and broadcasts it to a [128, GW]
            PSUM tile with rank-1 matmuls."""
            G = slice(GW * g, GW * (g + 1))
            for k in range(kt):
                nc.sync.dma_start(
                    out=zt[k][:, G], in_=zT_d[P * k : P * k + pdims[k], G]
                )
            sqs = [
                sqp.tile([pdims[k], GW], bf16, tag=f"sq{k}", name=f"sq{k}")
                for k in range(kt)
            ]
            for k in range(kt):
                nc.vector.tensor_mul(sqs[k], zt[k][:, G], zt[k][:, G])
            ps_ss = mps.tile([P, GW], f32, tag="ps", name="ps_ss")
            for c in range(swch):
                for k in range(kt):
                    nc.tensor.matmul(
                        ps_ss[0:1, CH * c : CH * (c + 1)],
                        ones_sb[: pdims[k]],
                        sqs[k][:, CH * c : CH * (c + 1)],
                        start=(k == 0),
                        stop=(k == kt - 1),
                    )
            sschunk = small.tile([1, GW], f32, tag="sschunk")
            nc.vector.tensor_copy(out=sschunk, in_=ps_ss[0:1, :])
            # DRAM round-trip to reshape [1, GW] -> [128, GW/128]
            nc.sync.dma_start(out=ssd[:, G], in_=sschunk)
            ss_pt = small.tile([P, tgw], f32, tag="ss_pt")
            nc.sync.dma_start(
                out=ss_pt, in_=ssd[0, G].rearrange("(t p) -> p t", p=P)
            )
            # inv = 1/sqrt(ss): Quake seed y0 = bits(0x5f3759df - (i >> 1))
            # then two Newton steps y *= 1.5 - 0.5*ss*y^2  (rel err ~5e-6)
            ii = small.tile([P, tgw], i32, tag="ii")
            nc.vector.tensor_scalar(
                out=ii, in0=ss_pt.bitcast(i32), scalar1=1, scalar2=None,
                op0=OP.arith_shift_right,
            )
            # K - t computed as (t ^ -1) + (K + 1); walrus disallows mixing
            # bitwise and arithmetic ops in one TENSOR_SCALAR, so two insts
            nc.vector.tensor_scalar(
                out=ii, in0=ii, scalar1=-1, scalar2=None, op0=OP.bitwise_xor
            )
            nc.vector.tensor_scalar(
                out=ii, in0=ii, scalar1=0x5F3759DF + 1, scalar2=None, op0=OP.add
            )
            y = ii.bitcast(f32)
            t_ = small.tile([P, tgw], f32, tag="t_")
            for _ in range(2):
                nc.vector.tensor_mul(t_, y, y)
                nc.vector.tensor_mul(t_, t_, ss_pt)
                nc.vector.tensor_scalar(
                    out=t_, in0=t_, scalar1=-0.5, scalar2=1.5,
                    op0=OP.mult, op1=OP.add,
                )
                nc.vector.tensor_mul(y, y, t_)
            nc.sync.dma_start(
                out=invd[0, G].rearrange("(t p) -> p t", p=P), in_=y
            )
            bc = bcp.tile([P, GW], f32, tag="bc", name="bc")
            nc.gpsimd.dma_start(out=bc, in_=invd[:, G].to_broadcast([P, GW]))
            for k in range(kt):
                nc.vector.tensor_mul(znb[k][:, G], zt[k][:, G], bc[: pdims[k]])

        def main_sweep(s):
            for m in range(mt):
                ps = mps.tile([P, GW], f32, tag="ps", name="ps")
                for k in range(kt):
                    for c in range(swch):
                        cols = slice(GW * s + CH * c, GW * s + CH * (c + 1))
                        nc.tensor.matmul(
                            ps[:, CH * c : CH * (c + 1)],
                            znb[k][:, P * m : P * (m + 1)],
                            znb[k][:, cols],
                            start=(k == 0),
                            stop=(k == kt - 1),
                        )
                w0 = n // 2 + P * m  # positive-pair window (local cols)
                if w0 // GW == s:
                    off = w0 % GW
                    junk = small.tile([P, P], f32, tag="junk")
                    nc.vector.scalar_tensor_tensor(
                        out=junk,
                        in0=ps[:, off : off + P],
                        scalar=2.0,
                        in1=ident_sb,
                        op0=OP.mult,
                        op1=OP.mult,
                        accum_out=poss[:, m : m + 1],
                    )
                nc.scalar.activation(
                    out=ps,
                    in_=ps,
                    func=FT.Exp,
                    scale=2.0,
                    accum_out=partials[:, m, s : s + 1],
                )

        # Interleave: each group's prologue is emitted two sweeps ahead so
        # no engine's in-order stream stalls on a later group's chain.
        for g in range(min(2, nsw)):
            prologue_group(g)
        for s in range(nsw):
            main_sweep(s)
            if s + 2 < nsw:
                prologue_group(s + 2)

        # ---- Per-row finalization ----
        for m in range(mt):
            S = small.tile([P, 1], f32, tag="S")
            nc.vector.tensor_reduce(
                out=S,
                in_=partials[:, m, :],
                axis=mybir.AxisListType.X,
                op=OP.add,
            )
            nc.vector.tensor_scalar_add(S, S, -EXP2)
            lse = small.tile([P, 1], f32, tag="lse")
            nc.scalar.activation(out=lse, in_=S, func=FT.Ln)
            nc.vector.tensor_tensor(
                out=out_sb[:, m : m + 1],
                in0=lse,
                in1=poss[:, m : m + 1],
                op=OP.subtract,
            )
        nc.sync.dma_start(out=out_d, in_=out_sb)

    nc.compile()
    return nc


def make_in_maps(z_i, z_j, n=N, rpc=RPC, ncores=NCORES):
    """Host-side sharding: rotated transposed embeddings per core."""
    z = np.concatenate(
        [np.asarray(z_i, dtype=np.float32), np.asarray(z_j, dtype=np.float32)],
        axis=0,
    )
    zT = np.ascontiguousarray(z.T)  # [D, N]
    ident = np.eye(128, dtype=np.float32)
    in_maps = []
    for c in range(ncores):
        zT_c = np.ascontiguousarray(np.roll(zT, -rpc * c, axis=1))
        in_maps.append({"zT": zT_c, "ident": ident})
    return in_maps


def gather_loss(results, n=N):
    """Host-side unshard: fp64 sum of all per-row (lse - pos) values / N."""
    total = 0.0
    for r in results:
        total += np.asarray(r["out"], dtype=np.float64).sum()
    return np.float32(total / n)


_PROGRAM_CACHE = {}


def kernel(z_i, z_j):
    _ensure_import_path()
    from concourse.bass_utils import run_bass_kernel_spmd

    key = (N, D, RPC)
    if key not in _PROGRAM_CACHE:
        _PROGRAM_CACHE[key] = build_program()
    nc = _PROGRAM_CACHE[key]
    in_maps = make_in_maps(z_i, z_j)
    results = run_bass_kernel_spmd(nc, in_maps, list(range(NCORES))).results
    return gather_loss(results)


if __name__ == "__main__":
    rng = np.random.default_rng(0)
    z_i = rng.standard_normal((B, D), dtype=np.float32)
    z_j = rng.standard_normal((B, D), dtype=np.float32)
    loss = kernel(z_i, z_j)
    print("loss:", loss)
